# revision 6
# baseline (speedup 1.0000x reference)
"""Trainium2 Bass kernel for nn_Pool_12919261627034 (topk_masking).

Per batch b:
  col_sum = h[b].sum(0)                      # [D]
  scores  = h[b] @ col_sum                   # [N]
  idx     = sorted(indices of K smallest)    # [K]
  new_h   = h[b][idx]                        # [K, D]
  new_adj = adj[b][idx]                      # [K, N]

Sharding: data-parallel over batch — one batch per NeuronCore (8 cores).

Device algorithm per core:
  1. Stream h into SBUF; accumulate col_sum with PE matmuls (ones vector).
  2. Broadcast col_sum to 128 partitions with a PE matmul.
  3. scores[c*128+p] via fused DVE multiply + accumulate (scalar_tensor_tensor
     accum_out), one op per 128-row chunk.
  4. Binary search (36 unrolled iterations) on the score value t = K-th
     smallest: count(scores <= mid) via tensor_scalar(is_le, accum_out) +
     all-ones matmul (gives the total replicated on all 128 partitions).
  5. Exact top-K selection with index tie-break at the threshold:
     keep = (s < t) | (s == t & global_eq_prefix <= K - count_lt).
  6. Compact selected indices, in ascending order, with the gpsimd
     sparse_gather instruction ([16, F] f-major layout).
  7. Indirect-DMA gather of the selected h rows (2KB) and adj rows (16KB),
     128 rows per instruction, streamed back out to DRAM.
"""

import numpy as np

B = 8
N = 4096
D = 512
K = 2048
P = 128
NCHUNK = N // P          # 32
NGROUP = 8               # h DMA groups (4 chunks = 1MB each)
NITER = 36               # binary search iterations
RANGE0 = 16384.0         # initial binary search range (scores are ~±8000)

_cache = {}


def _build_nc():
    import concourse.bacc as bacc
    import concourse.bass as bass
    import concourse.mybir as mybir
    import concourse.tile as tile

    dt = mybir.dt
    Alu = mybir.AluOpType
    f32 = dt.float32

    nc = bacc.Bacc("TRN2", target_bir_lowering=False)

    h = nc.dram_tensor("h", [N, D], f32, kind="ExternalInput")
    adj = nc.dram_tensor("adj", [N, N], f32, kind="ExternalInput")
    new_h = nc.dram_tensor("new_h", [K, D], f32, kind="ExternalOutput")
    new_adj = nc.dram_tensor("new_adj", [K, N], f32, kind="ExternalOutput")
    idx_dbg = nc.dram_tensor("idx_dbg", [K], dt.int32, kind="ExternalOutput")
    nf_dbg = nc.dram_tensor("nf_dbg", [1, 1], dt.uint32, kind="ExternalOutput")
    sc_dbg = nc.dram_tensor("sc_dbg", [N], f32, kind="ExternalOutput")
    thr_dbg = nc.dram_tensor("thr_dbg", [1, 4], f32, kind="ExternalOutput")

    # Constants (embedded in the NEFF).
    ones128_t = nc.inline_tensor(np.ones((P, P), np.float32), "ones128")
    # tri16[k, m] = 1 if k <= m  (inclusive prefix over partitions as lhsT)
    tri16_t = nc.inline_tensor(
        np.triu(np.ones((16, 16), np.float32), 0), "tri16"
    )
    # iota_p1[p, f] = (f*16 + p) + 1   (logical index n in f-major order, +1)
    iota_p1_t = nc.inline_tensor(
        (np.arange(N, dtype=np.float32).reshape(N // 16, 16).T + 1.0).copy(),
        "iota_p1",
    )

    with tile.TileContext(nc) as tc:
        with (
            tc.tile_pool(name="const", bufs=1) as constp,
            tc.tile_pool(name="hbig", bufs=1) as hbigp,
            tc.tile_pool(name="small", bufs=1) as smallp,
            tc.tile_pool(name="junk", bufs=2) as junkp,
            tc.tile_pool(name="adjrow", bufs=3) as adjp,
            tc.tile_pool(name="hrow", bufs=3) as hrowp,
            tc.tile_pool(name="dram", bufs=1, space="DRAM") as dramp,
            tc.tile_pool(name="psum1", bufs=1, space="PSUM") as psum1,
            tc.tile_pool(name="psum2", bufs=2, space="PSUM") as psum2,
        ):
            # ---- constants to SBUF ----
            ones128 = constp.tile([P, P], f32)
            nc.sync.dma_start(out=ones128[:], in_=ones128_t[:, :])
            tri16 = constp.tile([16, 16], f32)
            nc.sync.dma_start(out=tri16[:], in_=tri16_t[:, :])
            iota_p1 = constp.tile([16, N // 16], f32)
            nc.sync.dma_start(out=iota_p1[:], in_=iota_p1_t[:, :])

            # ---- 1. load h, accumulate col_sum on PE ----
            h_sb = hbigp.tile([P, NCHUNK * D], f32)
            cs_psum = psum1.tile([1, D], f32, space="PSUM")
            for g in range(NGROUP):
                cpg = NCHUNK // NGROUP  # chunks per group
                rows = cpg * P
                nc.sync.dma_start(
                    out=h_sb[:, g * cpg * D:(g + 1) * cpg * D].rearrange(
                        "p (c d) -> p c d", c=cpg
                    ),
                    in_=h[g * rows:(g + 1) * rows, :].rearrange(
                        "(c p) d -> p c d", p=P
                    ),
                )
                for i in range(cpg):
                    c = g * cpg + i
                    nc.tensor.matmul(
                        out=cs_psum[:],
                        lhsT=ones128[:, :1],
                        rhs=h_sb[:, c * D:(c + 1) * D],
                        start=(c == 0),
                        stop=(c == NCHUNK - 1),
                    )
            cs_sb = smallp.tile([1, D], f32)
            nc.scalar.copy(out=cs_sb[:], in_=cs_psum[:])

            # ---- 2. broadcast col_sum to 128 partitions ----
            csb_psum = psum1.tile([P, D], f32, space="PSUM")
            nc.tensor.matmul(
                out=csb_psum[:], lhsT=ones128[:1, :], rhs=cs_sb[:],
                start=True, stop=True,
            )
            cs_b = smallp.tile([P, D], f32)
            nc.scalar.copy(out=cs_b[:], in_=csb_psum[:])

            # ---- 3. scores[c*128+p] = h row . col_sum ----
            scores_sb = smallp.tile([P, NCHUNK], f32)
            for c in range(NCHUNK):
                prod_junk = junkp.tile([P, D], f32)
                nc.vector.scalar_tensor_tensor(
                    out=prod_junk[:],
                    in0=h_sb[:, c * D:(c + 1) * D],
                    scalar=1.0,
                    in1=cs_b[:],
                    op0=Alu.mult,
                    op1=Alu.mult,
                    accum_out=scores_sb[:, c:c + 1],
                )

            # ---- bounce scores to f-major [16, 256] layout (parallel with
            #      the binary search below) ----
            scores_bounce = dramp.tile([N], f32)
            nc.sync.dma_start(
                out=scores_bounce[:].rearrange("(c p) -> p c", p=P),
                in_=scores_sb[:],
            )
            s1625 = smallp.tile([16, N // 16], f32)
            nc.sync.dma_start(
                out=s1625[:],
                in_=scores_bounce[:].rearrange("(f p) -> p f", p=16),
            )

            # ---- 4. binary search for t = K-th smallest score ----
            lo = smallp.tile([P, 1], f32)
            hi = smallp.tile([P, 1], f32)
            mid = smallp.tile([P, 1], f32)
            part = smallp.tile([P, 1], f32)
            pred = smallp.tile([P, 1], dt.uint8)
            npred = smallp.tile([P, 1], dt.uint8)
            nc.vector.memset(lo[:], -RANGE0)
            nc.vector.memset(hi[:], RANGE0)
            for it in range(NITER):
                bs_junk = junkp.tile([P, NCHUNK], f32, tag="bsjunk")
                nc.vector.tensor_tensor(
                    out=mid[:], in0=lo[:], in1=hi[:], op=Alu.add
                )
                nc.vector.tensor_scalar_mul(mid[:], mid[:], 0.5)
                nc.vector.tensor_scalar(
                    out=bs_junk[:],
                    in0=scores_sb[:],
                    scalar1=mid[:],
                    scalar2=None,
                    op0=Alu.is_le,
                    op1=Alu.add,
                    accum_out=part[:],
                )
                cnt_psum = psum2.tile([P, 1], f32, space="PSUM", tag="cnt")
                nc.tensor.matmul(
                    out=cnt_psum[:], lhsT=ones128[:], rhs=part[:],
                    start=True, stop=True,
                )
                nc.vector.tensor_scalar(
                    out=pred[:], in0=cnt_psum[:], scalar1=float(K),
                    scalar2=None, op0=Alu.is_ge,
                )
                nc.vector.tensor_scalar(
                    out=npred[:], in0=cnt_psum[:], scalar1=float(K),
                    scalar2=None, op0=Alu.is_lt,
                )
                nc.vector.copy_predicated(out=hi[:], mask=pred[:], data=mid[:])
                nc.vector.copy_predicated(out=lo[:], mask=npred[:], data=mid[:])

            nc.sync.dma_start(
                out=sc_dbg.rearrange("(c p) -> p c", p=P), in_=scores_sb[:]
            )
            thr4 = smallp.tile([1, 4], f32)
            nc.vector.tensor_copy(out=thr4[:, 0:1], in_=hi[:1, :])
            nc.vector.tensor_copy(out=thr4[:, 1:2], in_=lo[:1, :])
            nc.vector.tensor_copy(out=thr4[:, 2:3], in_=part[:1, :])
            nc.vector.tensor_copy(out=thr4[:, 3:4], in_=mid[:1, :])
            nc.sync.dma_start(out=thr_dbg[:, :], in_=thr4[:])

            # ---- 5. exact selection mask with index tie-break ----
            thr16 = hi[:16, :1]
            F = N // 16  # 256
            mlt = smallp.tile([16, F], f32)
            red_lt = smallp.tile([16, 1], f32)
            nc.vector.tensor_scalar(
                out=mlt[:], in0=s1625[:], scalar1=thr16, scalar2=None,
                op0=Alu.is_lt, op1=Alu.add, accum_out=red_lt[:],
            )
            cnt16_psum = psum1.tile([16, 1], f32, space="PSUM")
            nc.tensor.matmul(
                out=cnt16_psum[:], lhsT=ones128[:16, :16],
                rhs=red_lt[:], start=True, stop=True,
            )
            # need = K - count_lt   (count of == t entries to keep)
            need16 = smallp.tile([16, 1], f32)
            nc.vector.tensor_scalar(
                out=need16[:], in0=cnt16_psum[:], scalar1=-1.0,
                scalar2=float(K), op0=Alu.mult, op1=Alu.add,
            )
            meq = smallp.tile([16, F], f32)
            nc.vector.tensor_scalar(
                out=meq[:], in0=s1625[:], scalar1=thr16, scalar2=None,
                op0=Alu.is_equal,
            )
            # global inclusive prefix of meq in f-major (logical n) order:
            #   colsum[f]  = sum_p meq[p, f]          (replicated, PE)
            #   colcum[f]  = inclusive scan_f colsum  (DVE scan)
            #   partial    = sum_{p'<=p} meq[p', f]   (PE, tri16 lhsT)
            #   prefix     = colcum - colsum + partial
            colsum_psum = psum1.tile([16, F], f32, space="PSUM")
            nc.tensor.matmul(
                out=colsum_psum[:], lhsT=ones128[:16, :16],
                rhs=meq[:], start=True, stop=True,
            )
            zeros16 = smallp.tile([16, F], f32)
            nc.vector.memset(zeros16[:], 0.0)
            colcum = smallp.tile([16, F], f32)
            nc.vector.tensor_tensor_scan(
                out=colcum[:], data0=zeros16[:], data1=colsum_psum[:],
                initial=0.0, op0=Alu.add, op1=Alu.add,
            )
            partial_psum = psum1.tile([16, F], f32, space="PSUM")
            nc.tensor.matmul(
                out=partial_psum[:], lhsT=tri16[:], rhs=meq[:],
                start=True, stop=True,
            )
            excl = smallp.tile([16, F], f32)
            nc.vector.tensor_tensor(
                out=excl[:], in0=colcum[:], in1=colsum_psum[:], op=Alu.subtract
            )
            prefix = smallp.tile([16, F], f32)
            nc.vector.tensor_tensor(
                out=prefix[:], in0=excl[:], in1=partial_psum[:], op=Alu.add
            )
            keepeq = smallp.tile([16, F], f32)
            nc.vector.tensor_scalar(
                out=keepeq[:], in0=prefix[:], scalar1=need16[:], scalar2=None,
                op0=Alu.is_le,
            )
            # select only where s == t:
            keepeq2 = smallp.tile([16, F], f32)
            nc.vector.tensor_tensor(
                out=keepeq2[:], in0=keepeq[:], in1=meq[:], op=Alu.mult
            )
            fmask = smallp.tile([16, F], f32)
            nc.vector.tensor_tensor(
                out=fmask[:], in0=mlt[:], in1=keepeq2[:], op=Alu.add
            )
            # masked_idx = fmask * (n+1) - 1   (selected -> n, else -1)
            masked = smallp.tile([16, F], f32)
            nc.vector.tensor_tensor(
                out=masked[:], in0=fmask[:], in1=iota_p1[:], op=Alu.mult
            )
            nc.vector.tensor_scalar(
                out=masked[:], in0=masked[:], scalar1=-1.0, scalar2=None,
                op0=Alu.add,
            )

            # ---- 6. compact to sorted index list ----
            compact = smallp.tile([16, K // 16], f32)
            nf = smallp.tile([1, 1], dt.uint32)
            nc.gpsimd.sparse_gather(
                out=compact[:], in_=masked[:], num_found=nf[:]
            )
            nc.sync.dma_start(out=nf_dbg[:, :], in_=nf[:])
            compact_i32 = smallp.tile([16, K // 16], dt.int32)
            nc.vector.tensor_copy(out=compact_i32[:], in_=compact[:])
            idx_bounce = dramp.tile([K], dt.int32)
            nc.sync.dma_start(
                out=idx_bounce[:].rearrange("(f p) -> p f", p=16),
                in_=compact_i32[:],
            )
            nc.sync.dma_start(
                out=idx_dbg.rearrange("(f p) -> p f", p=16),
                in_=compact_i32[:],
            )
            idx_sb = smallp.tile([P, K // P], dt.int32)
            nc.sync.dma_start(
                out=idx_sb[:],
                in_=idx_bounce[:].rearrange("(c j) -> j c", j=P),
            )

            # ---- 7. gather selected rows of adj and h ----
            for c in range(K // P):
                adjrow = adjp.tile([P, N], f32)
                nc.gpsimd.indirect_dma_start(
                    out=adjrow[:],
                    out_offset=None,
                    in_=adj[:, :],
                    in_offset=bass.IndirectOffsetOnAxis(
                        ap=idx_sb[:, c:c + 1], axis=0
                    ),
                )
                nc.sync.dma_start(
                    out=new_adj[c * P:(c + 1) * P, :], in_=adjrow[:]
                )
                hrow = hrowp.tile([P, D], f32)
                nc.gpsimd.indirect_dma_start(
                    out=hrow[:],
                    out_offset=None,
                    in_=h[:, :],
                    in_offset=bass.IndirectOffsetOnAxis(
                        ap=idx_sb[:, c:c + 1], axis=0
                    ),
                )
                nc.sync.dma_start(
                    out=new_h[c * P:(c + 1) * P, :], in_=hrow[:]
                )

    nc.compile()
    return nc


def _get_nc():
    if "nc" not in _cache:
        _cache["nc"] = _build_nc()
    return _cache["nc"]


def kernel(h, adj):
    from concourse.bass_utils import run_bass_kernel_spmd

    h = np.ascontiguousarray(np.asarray(h), dtype=np.float32)
    adj = np.ascontiguousarray(np.asarray(adj), dtype=np.float32)
    assert h.shape == (B, N, D) and adj.shape == (B, N, N)

    nc = _get_nc()
    in_maps = [{"h": h[b], "adj": adj[b]} for b in range(B)]
    res = run_bass_kernel_spmd(nc, in_maps, core_ids=list(range(B)))
    new_h = np.stack([res.results[b]["new_h"] for b in range(B)])
    new_adj = np.stack([res.results[b]["new_adj"] for b in range(B)])
    return new_h, new_adj
